# revision 20
# baseline (speedup 1.0000x reference)
"""Trainium2 Bass kernel for nn_MeshReduce (retrieval_knn).

Pipeline (reference semantics):
  h   = layernorm(node_features)                      [B*Nm, C]
  piv = knn_interp(h.reshape(B,Nm,C), pos_mesh, pos_piv, k=3)   [B, Npiv, C]
  out = knn_interp(piv, pos_piv, pos_mesh, k=3)                 [B, Nm, C]

Device strategy (8 NeuronCores, SPMD, two launches, no collectives):
  Launch 1 ("knn"): batch-independent. Computes top-3 neighbor indices +
    normalized inverse-d2 weights for BOTH interpolation directions.
    Queries are sharded 8 ways. Scores s = -d2 = 2*y.x - |x|^2 - |y|^2 are
    computed on the PE (K=4 matmul: rows -2y0,-2y1,-2y2,1 against rows
    x0,x1,x2,|x|^2) with the -|y|^2 bias applied during the PSUM->SBUF
    evacuation on the scalar engine. Top-8 values/indices per 128-query
    block via the DVE max / max_index instructions; multi-chunk candidate
    sets (encode: 40960 cands) are merged with a second max over the
    per-chunk top-8s, and indices recovered by re-running max_index per
    chunk against the merged top-8 (unmatched slots return 0xFFFF which
    acts as a +inf sentinel in a min-combine).
  Host glue: concatenates the idx/weight shards and re-marshals them into
    device-friendly layouts for launch 2 (pure data movement, no math).
  Launch 2 ("interp"): per core c: batch b=c//2, mesh-row half h=c%2.
    Gathers the (<=6144) needed node_feature rows via dma_gather (pair
    trick: 2-row 1KB elements so indices fit int16, select by parity),
    applies layernorm, computes piv (gamma/beta folded to the end since
    weights are normalized), writes piv to DRAM, then gathers piv rows for
    its 20480 mesh queries (10..20 dma_gathers round-robined over SWDGE
    queues) and combines with the decode weights.
"""

import functools

import numpy as np

import concourse.bacc as bacc
import concourse.bass as bass
import concourse.mybir as mybir
import concourse.tile as tile
from concourse import library_config
from concourse.bass_utils import run_bass_kernel_spmd

F32 = mybir.dt.float32
U16 = mybir.dt.uint16
I16 = mybir.dt.int16
AF = mybir.ActivationFunctionType
ALU = mybir.AluOpType
AX = mybir.AxisListType

P = 128
NCORES = 8

# Problem sizes (hardcoded per the harness contract).
B, NM, NP, C = 4, 40000, 2000, 128
NMP, NPP = 40960, 2048          # padded candidate counts (multiples of 512)
QE = NPP // NCORES              # 256 encode queries per core
QD = NMP // NCORES              # 5120 decode queries per core
EB, DB = QE // P, QD // P       # 2 / 40 query blocks per core
CH = 4096                       # encode candidate chunk (<= 16384 for max)
PADC = 100.0                    # padding coordinate (never a nearest neighbor)
EPS_W = 1e-16

# Launch 2 constants.
HROWS = NMP // 2                # 20480 mesh rows per half
GBLK = HROWS // P               # 160 query groups of 128
SB = 8                          # query groups per decode superblock
NSB = GBLK // SB                # 20 superblocks
NEIX = NPP * 3                  # 6144 encode gather rows
EIB = NEIX // P                 # 48 encode gather slots per partition


def _ceil_blocks(n, b):
    assert n % b == 0
    return n // b


@functools.cache
def _knn_program():
    nc = bacc.Bacc(None)
    ce4 = nc.declare_dram_parameter("ce4", [4, NMP], F32, isOutput=False)
    cd4 = nc.declare_dram_parameter("cd4", [4, NPP], F32, isOutput=False)
    qe4 = nc.declare_dram_parameter("qe4", [4, QE], F32, isOutput=False)
    qen = nc.declare_dram_parameter("qen", [P, EB], F32, isOutput=False)
    qd4 = nc.declare_dram_parameter("qd4", [4, QD], F32, isOutput=False)
    qdn = nc.declare_dram_parameter("qdn", [P, DB], F32, isOutput=False)
    eio = nc.declare_dram_parameter("ei", [P, EB, 3], F32, isOutput=True)
    ewo = nc.declare_dram_parameter("ew", [P, EB, 3], F32, isOutput=True)
    dio = nc.declare_dram_parameter("di", [P, DB, 3], U16, isOutput=True)
    dwo = nc.declare_dram_parameter("dw", [P, DB, 3], F32, isOutput=True)

    with tile.TileContext(nc) as tc:
        _knn_phase(tc, ce4, qe4, qen, EB, NMP, eio, ewo, multi=True, name="e")
        _knn_phase(tc, cd4, qd4, qdn, DB, NPP, dio, dwo, multi=False, name="d")
    nc.finalize()
    return nc


def _knn_phase(tc, cand, q4, qn, nblk, ncand, idx_out, w_out, multi, name):
    """Top-3 + normalized weights for nblk*128 queries over ncand candidates."""
    nc = tc.nc
    with (
        tc.tile_pool(name=f"{name}pp", bufs=1) as pp,
        tc.tile_pool(name=f"{name}sp", bufs=3) as sp,
        tc.tile_pool(name=f"{name}ps", bufs=6, space="PSUM") as psp,
    ):
        chsz = CH if multi else ncand
        nch = ncand // chsz

        vals = pp.tile([P, nblk, 8], F32, tag="vals")
        ytile = pp.tile([P, nblk], F32, tag="yt")
        nc.sync.dma_start(ytile[:], qn[:])
        lq = pp.tile([4, nblk * P], F32, tag="lq")
        nc.sync.dma_start(lq[:], q4[:])

        if multi:
            idxf = pp.tile([P, nblk, 3], F32, tag="idxf")
        else:
            dist = pp.tile([P, nblk, 8], U16, tag="dist")

        def score_chunk(b, c0, sz, keep):
            rhs = sp.tile([4, sz], F32, tag="rhs")
            nc.sync.dma_start(rhs[:], cand[:, c0 : c0 + sz])
            sc = sp.tile([P, sz], F32, tag="sc")
            for j in range(0, sz, 512):
                ps = psp.tile([P, 512], F32, tag="ps")
                nc.tensor.matmul(
                    ps[:],
                    lhsT=lq[:, b * P : (b + 1) * P],
                    rhs=rhs[:, j : j + 512],
                    start=True,
                    stop=True,
                )
                # out = -(psum) - |y|^2 = 2*y.x - |x|^2 - |y|^2 = -d2
                nc.scalar.activation(
                    sc[:, j : j + 512],
                    ps[:],
                    AF.Identity,
                    bias=ytile[:, b : b + 1],
                    scale=-1.0,
                )
            return sc

        for b in range(nblk):
            if multi:
                chv = sp.tile([P, nch * 8], F32, tag="chv")
                for ci in range(nch):
                    sc = score_chunk(b, ci * chsz, chsz, keep=False)
                    nc.vector.max(chv[:, ci * 8 : (ci + 1) * 8], sc[:])
                nc.vector.max(vals[:, b, :], chv[:])
                for ci in range(nch):
                    sc = score_chunk(b, ci * chsz, chsz, keep=False)
                    iu = sp.tile([P, 8], U16, tag="iu")
                    nc.vector.max_index(iu[:], vals[:, b, :], sc[:])
                    fi = sp.tile([P, 8], F32, tag="fi")
                    nc.vector.tensor_copy(fi[:], iu[:])
                    if ci == 0:
                        nc.vector.tensor_copy(idxf[:, b, :], fi[:, 0:3])
                    else:
                        nc.vector.tensor_scalar(
                            fi[:], fi[:], float(ci * chsz), None, op0=ALU.add
                        )
                        nc.vector.tensor_tensor(
                            idxf[:, b, :], idxf[:, b, :], fi[:, 0:3], op=ALU.min
                        )
            else:
                sc = score_chunk(b, 0, ncand, keep=True)
                nc.vector.max(vals[:, b, :], sc[:])
                nc.vector.max_index(dist[:, b, :], vals[:, b, :], sc[:])

        # Batched weight computation: w = 1/clip(d2, eps); normalize.
        wp = pp.tile([P, nblk, 3], F32, tag="wp")
        nc.vector.tensor_scalar(
            wp[:], vals[:, :, 0:3], -1.0, EPS_W, op0=ALU.mult, op1=ALU.max
        )
        nc.vector.reciprocal(wp[:], wp[:])
        ws = pp.tile([P, nblk], F32, tag="ws")
        nc.vector.reduce_sum(ws[:], wp[:], axis=AX.X)
        nc.vector.reciprocal(ws[:], ws[:])
        wn = pp.tile([P, nblk, 3], F32, tag="wn")
        nc.vector.tensor_tensor(
            wn[:], wp[:], ws[:, :, None].to_broadcast([P, nblk, 3]), op=ALU.mult
        )
        nc.sync.dma_start(w_out[:], wn[:])
        if multi:
            nc.sync.dma_start(idx_out[:], idxf[:])
        else:
            nc.sync.dma_start(idx_out[:], dist[:, :, 0:3])


NQUEUES = 4  # ucode MAX_SWDGE_QUEUES

# Exec times (ns) of the two launches from the most recent kernel() call,
# populated when profiling is enabled (BASS_TRACE=1); None entries otherwise.
LAST_EXEC_NS = []


@functools.cache
def _interp_program():
    nc = bacc.Bacc(None, num_swdge_queues=NQUEUES)
    nf = nc.declare_dram_parameter("nf", [NM, C], F32, isOutput=False)
    gb2 = nc.declare_dram_parameter("gb2", [2, C], F32, isOutput=False)
    eix = nc.declare_dram_parameter("eix", [P, NEIX // 16], I16, isOutput=False)
    epar = nc.declare_dram_parameter("epar", [P, EIB], F32, isOutput=False)
    ewm = nc.declare_dram_parameter("ewm", [P, EIB], F32, isOutput=False)
    dix = nc.declare_dram_parameter("dix", [P, HROWS * 3 // 16], I16, isOutput=False)
    dwm = nc.declare_dram_parameter("dwm", [P, GBLK * 3], F32, isOutput=False)
    outd = nc.declare_dram_parameter("outd", [P, GBLK * C], F32, isOutput=True)

    with tile.TileContext(nc) as tc:
        with (
            tc.tile_pool(name="pp", bufs=1) as pp,
            tc.tile_pool(name="dr", bufs=1, space="DRAM") as dp,
            tc.tile_pool(name="psp", bufs=2, space="PSUM") as psp,
        ):
            # gpsimd ucode library containing DMAGatherAnt
            nc.gpsimd.load_library(library_config.mlp)

            # ---- small inputs
            eix_sb = pp.tile([P, NEIX // 16], I16, tag="eix")
            nc.sync.dma_start(eix_sb[:], eix[:])
            epar_sb = pp.tile([P, EIB], F32, tag="epar")
            nc.sync.dma_start(epar_sb[:], epar[:])
            ewm_sb = pp.tile([P, EIB], F32, tag="ewm")
            nc.sync.dma_start(ewm_sb[:], ewm[:])
            dix_sb = pp.tile([P, HROWS * 3 // 16], I16, tag="dix")
            nc.sync.dma_start(dix_sb[:], dix[:])
            dwm_sb = pp.tile([P, GBLK * 3], F32, tag="dwm")
            nc.sync.dma_start(dwm_sb[:], dwm[:])
            g1 = pp.tile([1, C], F32, tag="g1")
            nc.sync.dma_start(g1[:], gb2[0:1, :])
            b1 = pp.tile([1, C], F32, tag="b1")
            nc.sync.dma_start(b1[:], gb2[1:2, :])

            # ---- gamma/beta broadcast across partitions via K=1 matmul
            ones1 = pp.tile([1, P], F32, tag="ones1")
            nc.vector.memset(ones1[:], 1.0)
            gbc = pp.tile([P, C], F32, tag="gbc")
            bbc = pp.tile([P, C], F32, tag="bbc")
            for src, dst in ((g1, gbc), (b1, bbc)):
                psb = psp.tile([P, C], F32, tag="psb")
                nc.tensor.matmul(
                    psb[:], lhsT=ones1[:], rhs=src[:],
                    start=True, stop=True,
                )
                nc.scalar.activation(dst[:], psb[:], AF.Copy)

            piv_p = pp.tile([P, 16, C], F32, tag="pivp")

            # ---- phase E: gather node features (pairs), layernorm, combine
            with tc.tile_pool(name="pe", bufs=1) as pe:
                g2 = pe.tile([P, EIB, 2 * C], F32, tag="g2")
                ncalls = 8
                per = NEIX // ncalls            # 768 idx per call
                nf_pairs = nf[:].rearrange("(n two) c -> n (two c)", two=2)
                for ci in range(ncalls):
                    nc.gpsimd.dma_gather(
                        g2[:, ci * (EIB // ncalls) : (ci + 1) * (EIB // ncalls), :],
                        nf_pairs,
                        eix_sb[:, ci * (per // 16) : (ci + 1) * (per // 16)],
                        num_idxs=per,
                        num_idxs_reg=per,
                        elem_size=2 * C,
                        queue_num=ci % NQUEUES,
                    )
                # ge = (1-par)*g2_lo + par*g2_hi  (exact: par is 0.0/1.0)
                parc = pe.tile([P, EIB], F32, tag="parc")
                nc.vector.tensor_scalar(
                    parc[:], epar_sb[:], -1.0, 1.0, op0=ALU.mult, op1=ALU.add
                )
                ge = pe.tile([P, EIB, C], F32, tag="ge")
                tsel = pe.tile([P, EIB, C], F32, tag="tsel")
                nc.vector.tensor_tensor(
                    ge[:], g2[:, :, 0:C],
                    parc[:, :, None].to_broadcast([P, EIB, C]), op=ALU.mult,
                )
                nc.vector.tensor_tensor(
                    tsel[:], g2[:, :, C : 2 * C],
                    epar_sb[:, :, None].to_broadcast([P, EIB, C]), op=ALU.mult,
                )
                nc.vector.tensor_tensor(ge[:], ge[:], tsel[:], op=ALU.add)
                # layernorm stats per gathered row
                mu = pe.tile([P, EIB], F32, tag="mu")
                nc.vector.reduce_sum(mu[:], ge[:], axis=AX.X)
                nc.vector.tensor_scalar(mu[:], mu[:], 1.0 / C, None, op0=ALU.mult)
                nc.vector.tensor_tensor(
                    ge[:], ge[:], mu[:, :, None].to_broadcast([P, EIB, C]),
                    op=ALU.subtract,
                )
                sq = pe.tile([P, EIB, C], F32, tag="sq")
                nc.scalar.square(sq[:], ge[:])
                vs = pe.tile([P, EIB], F32, tag="vs")
                nc.vector.reduce_sum(vs[:], sq[:], axis=AX.X)
                nc.vector.tensor_scalar(
                    vs[:], vs[:], 1.0 / C, 1e-5, op0=ALU.mult, op1=ALU.add
                )
                nc.scalar.sqrt(vs[:], vs[:])
                nc.vector.reciprocal(vs[:], vs[:])
                # fold rstd * weight into one multiplier
                nc.vector.tensor_tensor(vs[:], vs[:], ewm_sb[:], op=ALU.mult)
                nc.vector.tensor_tensor(
                    ge[:], ge[:], vs[:, :, None].to_broadcast([P, EIB, C]),
                    op=ALU.mult,
                )
                ger = ge[:].rearrange("p (u k) c -> p u k c", k=3)
                nc.vector.tensor_tensor(
                    piv_p[:], ger[:, :, 0, :], ger[:, :, 1, :], op=ALU.add
                )
                nc.vector.tensor_tensor(
                    piv_p[:], piv_p[:], ger[:, :, 2, :], op=ALU.add
                )
                # gamma/beta (weights sum to 1, so affine folds to the end)
                nc.vector.tensor_tensor(
                    piv_p[:], piv_p[:],
                    gbc[:, None, :].to_broadcast([P, 16, C]), op=ALU.mult,
                )
                nc.vector.tensor_tensor(
                    piv_p[:], piv_p[:],
                    bbc[:, None, :].to_broadcast([P, 16, C]), op=ALU.add,
                )

            piv_d = dp.tile([NPP, C], F32, tag="pivd")
            nc.sync.dma_start(
                piv_d[:].rearrange("(p u) c -> p u c", p=P), piv_p[:]
            )

            # ---- phase D: gather piv rows, combine with decode weights
            with tc.tile_pool(name="pd", bufs=2) as pd:
                for s in range(NSB):
                    g = pd.tile([P, 3 * SB, C], F32, tag="g")
                    nidx = 3 * SB * P          # 3072
                    nc.gpsimd.dma_gather(
                        g[:],
                        piv_d[:],
                        dix_sb[:, s * (nidx // 16) : (s + 1) * (nidx // 16)],
                        num_idxs=nidx,
                        num_idxs_reg=nidx,
                        elem_size=C,
                        queue_num=s % NQUEUES,
                        single_packet=False,  # >1024 descriptors per call
                    )
                    nc.vector.tensor_tensor(
                        g[:], g[:],
                        dwm_sb[:, s * 3 * SB : (s + 1) * 3 * SB][:, :, None]
                        .to_broadcast([P, 3 * SB, C]),
                        op=ALU.mult,
                    )
                    gr = g[:].rearrange("p (u k) c -> p u k c", k=3)
                    o = pd.tile([P, SB, C], F32, tag="o")
                    nc.vector.tensor_tensor(
                        o[:], gr[:, :, 0, :], gr[:, :, 1, :], op=ALU.add
                    )
                    nc.vector.tensor_tensor(o[:], o[:], gr[:, :, 2, :], op=ALU.add)
                    nc.sync.dma_start(
                        outd[:, s * SB * C : (s + 1) * SB * C], o[:]
                    )
    nc.finalize()
    return nc


# ---------------------------------------------------------------------------
# Host-side marshalling
# ---------------------------------------------------------------------------

def _pad_coords(pos, n):
    out = np.full((n, 3), PADC, np.float32)
    out[: pos.shape[0]] = pos
    return out


def _aug_cand(pos):
    # rows x0, x1, x2, |x|^2
    return np.concatenate(
        [pos.T, (pos * pos).sum(-1)[None, :]], axis=0
    ).astype(np.float32)


def _aug_query(pos):
    # rows -2*y0, -2*y1, -2*y2, 1
    return np.concatenate(
        [(-2.0 * pos).T, np.ones((1, pos.shape[0]), np.float32)], axis=0
    ).astype(np.float32)


def _wrap16(x_stream):
    """[N] stream (position j) -> [128, N//16] wrapped int16 (replicated x8)."""
    w16 = x_stream.reshape(-1, 16).T  # [16, N//16]
    return np.ascontiguousarray(np.tile(w16, (8, 1)).astype(np.int16))


def _slot_layout(idx_qk):
    """[Q, 3] per-query data -> [P, 3*Q/P] slot layout: out[p, 3g+k] = in[g*128+p, k]."""
    q = idx_qk.shape[0]
    g = q // P
    return np.ascontiguousarray(
        idx_qk.reshape(g, P, 3).transpose(1, 0, 2).reshape(P, 3 * g)
    )


def launch1_inputs(pm, pv):
    """Per-core input dicts for the knn launch."""
    cm = _pad_coords(pm, NMP)
    cp = _pad_coords(pv, NPP)
    ce4 = _aug_cand(cm)
    cd4 = _aug_cand(cp)
    qe4g = _aug_query(cp)
    qeng = (-(cp * cp).sum(-1)).astype(np.float32)
    qd4g = _aug_query(cm)
    qdng = (-(cm * cm).sum(-1)).astype(np.float32)

    in1 = []
    for c in range(NCORES):
        in1.append(
            dict(
                ce4=ce4,
                cd4=cd4,
                qe4=np.ascontiguousarray(qe4g[:, c * QE : (c + 1) * QE]),
                qen=np.ascontiguousarray(
                    qeng[c * QE : (c + 1) * QE].reshape(EB, P).T
                ),
                qd4=np.ascontiguousarray(qd4g[:, c * QD : (c + 1) * QD]),
                qdn=np.ascontiguousarray(
                    qdng[c * QD : (c + 1) * QD].reshape(DB, P).T
                ),
            )
        )
    return in1


def launch1_post(r1):
    """Assemble per-core knn outputs into global idx/weight tables."""
    enc_idx = np.zeros((NPP, 3), np.int64)
    enc_w = np.zeros((NPP, 3), np.float32)
    dec_idx = np.zeros((NMP, 3), np.int64)
    dec_w = np.zeros((NMP, 3), np.float32)
    for c in range(NCORES):
        ei = np.asarray(r1[c]["ei"])  # [P, EB, 3] f32
        ew = np.asarray(r1[c]["ew"])
        di = np.asarray(r1[c]["di"])  # [P, DB, 3] u16
        dw = np.asarray(r1[c]["dw"])
        enc_idx[c * QE : (c + 1) * QE] = np.rint(
            ei.transpose(1, 0, 2).reshape(QE, 3)
        ).astype(np.int64)
        enc_w[c * QE : (c + 1) * QE] = ew.transpose(1, 0, 2).reshape(QE, 3)
        dec_idx[c * QD : (c + 1) * QD] = (
            di.transpose(1, 0, 2).reshape(QD, 3).astype(np.int64)
        )
        dec_w[c * QD : (c + 1) * QD] = dw.transpose(1, 0, 2).reshape(QD, 3)
    return enc_idx, enc_w, dec_idx, dec_w


def launch2_inputs(nf, gam, bet, enc_idx, enc_w, dec_idx, dec_w):
    """Per-core input dicts for the interp launch."""
    # Padded pivots may select padded mesh candidates (idx >= NM); their piv
    # rows are only ever consumed by padded decode queries whose outputs are
    # discarded, so point them at row 0.
    enc_idx = np.where(enc_idx >= NM, 0, enc_idx)
    # Encode gather marshalling (identical on the two cores of a batch).
    # Slot (p, i=3u+k) <- pivot 16p+u, neighbor k.
    piv_ids = (16 * np.arange(P)[:, None] + np.arange(16)[None, :]).reshape(-1)
    e_idx_slot = enc_idx[piv_ids].reshape(P, EIB)          # [P, 48]
    e_w_slot = enc_w[piv_ids].reshape(P, EIB).astype(np.float32)
    e_pair_slot = (e_idx_slot >> 1).astype(np.int16)
    e_par_slot = (e_idx_slot & 1).astype(np.float32)
    # stream j = i*128 + p -> value slot[p, i]
    eix_h = _wrap16(e_pair_slot.T.reshape(-1))
    gb2_h = np.ascontiguousarray(np.stack([gam, bet]).astype(np.float32))

    in2 = []
    for c in range(NCORES):
        b, h = divmod(c, 2)
        rows = slice(h * HROWS, (h + 1) * HROWS)
        d_idx_slot = _slot_layout(dec_idx[rows]).astype(np.int16)  # [P, 480]
        d_w_slot = _slot_layout(dec_w[rows]).astype(np.float32)
        dix_h = _wrap16(d_idx_slot.T.reshape(-1))
        in2.append(
            dict(
                nf=np.ascontiguousarray(nf[b * NM : (b + 1) * NM]),
                gb2=gb2_h,
                eix=eix_h,
                epar=np.ascontiguousarray(e_par_slot),
                ewm=np.ascontiguousarray(e_w_slot),
                dix=dix_h,
                dwm=d_w_slot,
            )
        )
    return in2


def launch2_post(r2):
    out = np.empty((B * NM, C), np.float32)
    for c in range(NCORES):
        b, h = divmod(c, 2)
        od = np.asarray(r2[c]["outd"]).reshape(P, GBLK, C)
        block = od.transpose(1, 0, 2).reshape(HROWS, C)
        lo = h * HROWS
        n_real = min(NM - lo, HROWS)
        out[b * NM + lo : b * NM + lo + n_real] = block[:n_real]
    return out


def kernel(node_features, gamma, beta, position_mesh, position_pivotal, batch_size):
    nf = np.ascontiguousarray(np.asarray(node_features, dtype=np.float32))
    gam = np.asarray(gamma, dtype=np.float32).reshape(C)
    bet = np.asarray(beta, dtype=np.float32).reshape(C)
    pm = np.asarray(position_mesh, dtype=np.float32)
    pv = np.asarray(position_pivotal, dtype=np.float32)
    bsz = int(batch_size)
    assert bsz == B and pm.shape[0] == NM and pv.shape[0] == NP

    in1 = launch1_inputs(pm, pv)
    rr1 = run_bass_kernel_spmd(_knn_program(), in1, list(range(NCORES)))
    enc_idx, enc_w, dec_idx, dec_w = launch1_post(rr1.results)

    in2 = launch2_inputs(nf, gam, bet, enc_idx, enc_w, dec_idx, dec_w)
    rr2 = run_bass_kernel_spmd(_interp_program(), in2, list(range(NCORES)))
    LAST_EXEC_NS.clear()
    LAST_EXEC_NS.extend([rr1.exec_time_ns, rr2.exec_time_ns])
    return launch2_post(rr2.results)


# revision 29
# speedup vs baseline: 1.2396x; 1.2396x over previous
"""Trainium2 Bass kernel for nn_MeshReduce (retrieval_knn).

Pipeline (reference semantics):
  h   = layernorm(node_features)                      [B*Nm, C]
  piv = knn_interp(h.reshape(B,Nm,C), pos_mesh, pos_piv, k=3)   [B, Npiv, C]
  out = knn_interp(piv, pos_piv, pos_mesh, k=3)                 [B, Nm, C]

Device strategy (8 NeuronCores, SPMD, two launches, no collectives):
  Launch 1 ("knn"): batch-independent. Computes top-3 neighbor indices +
    normalized inverse-d2 weights for BOTH interpolation directions.
    Queries are sharded 8 ways. Scores s = -d2 = 2*y.x - |x|^2 - |y|^2 are
    computed on the PE (K=4 matmul: rows -2y0,-2y1,-2y2,1 against rows
    x0,x1,x2,|x|^2) with the -|y|^2 bias applied during the PSUM->SBUF
    evacuation on the scalar engine. Top-8 values/indices per 128-query
    block via the DVE max / max_index instructions; multi-chunk candidate
    sets (encode: 40960 cands) are merged with a second max over the
    per-chunk top-8s, and indices recovered by re-running max_index per
    chunk against the merged top-8 (unmatched slots return 0xFFFF which
    acts as a +inf sentinel in a min-combine).
  Host glue: concatenates the idx/weight shards and re-marshals them into
    device-friendly layouts for launch 2 (pure data movement, no math).
  Launch 2 ("interp"): per core c: batch b=c//2, mesh-row half h=c%2.
    Gathers the (<=6144) needed node_feature rows via dma_gather (pair
    trick: 2-row 1KB elements so indices fit int16, select by parity),
    applies layernorm, computes piv (gamma/beta folded to the end since
    weights are normalized), writes piv to DRAM, then gathers piv rows for
    its 20480 mesh queries (10..20 dma_gathers round-robined over SWDGE
    queues) and combines with the decode weights.
"""

import functools

import numpy as np

import concourse.bacc as bacc
import concourse.bass as bass
import concourse.mybir as mybir
import concourse.tile as tile
from concourse import library_config
from concourse.bass_utils import run_bass_kernel_spmd

F32 = mybir.dt.float32
BF16 = mybir.dt.bfloat16
U16 = mybir.dt.uint16
I16 = mybir.dt.int16
AF = mybir.ActivationFunctionType
ALU = mybir.AluOpType
AX = mybir.AxisListType

P = 128
NCORES = 8

# Problem sizes (hardcoded per the harness contract).
B, NM, NP, C = 4, 40000, 2000, 128
NMP, NPP = 40960, 2048          # padded candidate counts (multiples of 512)
QE = NPP // NCORES              # 256 encode queries per core
QD = NMP // NCORES              # 5120 decode queries per core
EB, DB = QE // P, QD // P       # 2 / 40 query blocks per core
CH = 4096                       # encode candidate chunk (<= 16384 for max)
PADC = 100.0                    # padding coordinate (never a nearest neighbor)
EPS_W = 1e-16

# Launch 2 constants.
HROWS = NMP // 2                # 20480 mesh rows per half
GBLK = HROWS // P               # 160 query groups of 128
SB = 8                          # query groups per decode superblock
NSB = GBLK // SB                # 20 superblocks
NEIX = NPP * 3                  # 6144 encode gather rows
EIB = NEIX // P                 # 48 encode gather slots per partition


def _ceil_blocks(n, b):
    assert n % b == 0
    return n // b


@functools.cache
def _knn_program():
    nc = bacc.Bacc(None)
    ce4 = nc.declare_dram_parameter("ce4", [4, NMP], F32, isOutput=False)
    cd4 = nc.declare_dram_parameter("cd4", [4, NPP], F32, isOutput=False)
    qe4 = nc.declare_dram_parameter("qe4", [4, QE], F32, isOutput=False)
    qen = nc.declare_dram_parameter("qen", [P, EB], F32, isOutput=False)
    qd4 = nc.declare_dram_parameter("qd4", [4, QD], F32, isOutput=False)
    qdn = nc.declare_dram_parameter("qdn", [P, DB], F32, isOutput=False)
    eio = nc.declare_dram_parameter("ei", [P, EB, 3], F32, isOutput=True)
    ewo = nc.declare_dram_parameter("ew", [P, EB, 3], F32, isOutput=True)
    dio = nc.declare_dram_parameter("di", [P, DB, 3], U16, isOutput=True)
    dwo = nc.declare_dram_parameter("dw", [P, DB, 3], F32, isOutput=True)

    with tile.TileContext(nc) as tc:
        _knn_phase(tc, ce4, qe4, qen, EB, NMP, eio, ewo, multi=True, name="e")
        _knn_phase(tc, cd4, qd4, qdn, DB, NPP, dio, dwo, multi=False, name="d")
    nc.finalize()
    return nc


def _knn_phase(tc, cand, q4, qn, nblk, ncand, idx_out, w_out, multi, name):
    """Top-3 + normalized weights for nblk*128 queries over ncand candidates."""
    nc = tc.nc
    with (
        tc.tile_pool(name=f"{name}pp", bufs=1) as pp,
        tc.tile_pool(name=f"{name}sp", bufs=4) as sp,
        tc.tile_pool(name=f"{name}ps", bufs=8, space="PSUM") as psp,
    ):
        chsz = CH if multi else ncand
        nch = ncand // chsz

        vals = pp.tile([P, nblk, 8], F32, tag="vals")
        ytile = pp.tile([P, nblk], F32, tag="yt")
        nc.sync.dma_start(ytile[:], qn[:])
        lq = pp.tile([4, nblk * P], F32, tag="lq")
        nc.sync.dma_start(lq[:], q4[:])

        if multi:
            idxf = pp.tile([P, nblk, 3], F32, tag="idxf")
        else:
            dist = pp.tile([P, nblk, 8], U16, tag="dist")

        def score_chunk(b, c0, sz, tag="sc", pool=None):
            sc = (pool or sp).tile([P, sz], F32, tag=tag)
            for j in range(0, sz, 512):
                rhs = sp.tile([4, 512], F32, tag="rhs")
                nc.sync.dma_start(rhs[:], cand[:, c0 + j : c0 + j + 512])
                ps = psp.tile([P, 512], F32, tag="ps")
                nc.tensor.matmul(
                    ps[:],
                    lhsT=lq[:, b * P : (b + 1) * P],
                    rhs=rhs[:],
                    start=True,
                    stop=True,
                )
                # out = -(psum) - |y|^2 = 2*y.x - |x|^2 - |y|^2 = -d2
                nc.scalar.activation(
                    sc[:, j : j + 512],
                    ps[:],
                    AF.Identity,
                    bias=ytile[:, b : b + 1],
                    scale=-1.0,
                )
            return sc

        keep_cm = tc.tile_pool(name=f"{name}kp", bufs=1) if multi else None
        keep = keep_cm.__enter__() if keep_cm is not None else None
        for b in range(nblk):
            if multi:
                # Keep all candidate-chunk score tiles alive so the index
                # pass re-reads them instead of recomputing (halves ACT/PE).
                chv = sp.tile([P, nch * 8], F32, tag="chv")
                scs = []
                for ci in range(nch):
                    sc = score_chunk(b, ci * chsz, chsz, tag=f"sc{ci}", pool=keep)
                    scs.append(sc)
                    nc.vector.max(chv[:, ci * 8 : (ci + 1) * 8], sc[:])
                nc.vector.max(vals[:, b, :], chv[:])
                for ci in range(nch):
                    iu = sp.tile([P, 8], U16, tag="iu")
                    nc.vector.max_index(iu[:], vals[:, b, :], scs[ci][:])
                    fi = sp.tile([P, 8], F32, tag="fi")
                    nc.vector.tensor_copy(fi[:], iu[:])
                    if ci == 0:
                        nc.vector.tensor_copy(idxf[:, b, :], fi[:, 0:3])
                    else:
                        nc.vector.tensor_scalar(
                            fi[:], fi[:], float(ci * chsz), None, op0=ALU.add
                        )
                        nc.vector.tensor_tensor(
                            idxf[:, b, :], idxf[:, b, :], fi[:, 0:3], op=ALU.min
                        )
            else:
                sc = score_chunk(b, 0, ncand)
                nc.vector.max(vals[:, b, :], sc[:])
                nc.vector.max_index(dist[:, b, :], vals[:, b, :], sc[:])
        if keep_cm is not None:
            keep_cm.__exit__(None, None, None)

        # Batched weight computation: w = 1/clip(d2, eps); normalize.
        wp = pp.tile([P, nblk, 3], F32, tag="wp")
        nc.vector.tensor_scalar(
            wp[:], vals[:, :, 0:3], -1.0, EPS_W, op0=ALU.mult, op1=ALU.max
        )
        nc.vector.reciprocal(wp[:], wp[:])
        ws = pp.tile([P, nblk], F32, tag="ws")
        nc.vector.reduce_sum(ws[:], wp[:], axis=AX.X)
        nc.vector.reciprocal(ws[:], ws[:])
        wn = pp.tile([P, nblk, 3], F32, tag="wn")
        nc.vector.tensor_tensor(
            wn[:], wp[:], ws[:, :, None].to_broadcast([P, nblk, 3]), op=ALU.mult
        )
        nc.sync.dma_start(w_out[:], wn[:])
        if multi:
            nc.sync.dma_start(idx_out[:], idxf[:])
        else:
            nc.sync.dma_start(idx_out[:], dist[:, :, 0:3])


NQUEUES = 4  # ucode MAX_SWDGE_QUEUES

# Exec times (ns) of the two launches from the most recent kernel() call,
# populated when profiling is enabled (BASS_TRACE=1); None entries otherwise.
LAST_EXEC_NS = []


@functools.cache
def _interp_program():
    nc = bacc.Bacc(None, num_swdge_queues=NQUEUES)
    nf = nc.declare_dram_parameter("nf", [NM, C], F32, isOutput=False)
    gb2 = nc.declare_dram_parameter("gb2", [2, C], F32, isOutput=False)
    eix = nc.declare_dram_parameter("eix", [P, NEIX // 16], I16, isOutput=False)
    epar = nc.declare_dram_parameter("epar", [P, EIB], F32, isOutput=False)
    ewm = nc.declare_dram_parameter("ewm", [P, EIB], F32, isOutput=False)
    dix = nc.declare_dram_parameter("dix", [P, HROWS * 3 // 16], I16, isOutput=False)
    dwm = nc.declare_dram_parameter("dwm", [P, GBLK * 3], F32, isOutput=False)
    outd = nc.declare_dram_parameter("outd", [P, GBLK * C], F32, isOutput=True)

    with tile.TileContext(nc) as tc:
        with (
            tc.tile_pool(name="pp", bufs=1) as pp,
            tc.tile_pool(name="dr", bufs=1, space="DRAM") as dp,
            tc.tile_pool(name="psp", bufs=2, space="PSUM") as psp,
        ):
            # gpsimd ucode library containing DMAGatherAnt
            nc.gpsimd.load_library(library_config.mlp)

            # ---- small inputs
            eix_sb = pp.tile([P, NEIX // 16], I16, tag="eix")
            nc.sync.dma_start(eix_sb[:], eix[:])
            epar_sb = pp.tile([P, EIB], F32, tag="epar")
            nc.sync.dma_start(epar_sb[:], epar[:])
            ewm_sb = pp.tile([P, EIB], F32, tag="ewm")
            nc.sync.dma_start(ewm_sb[:], ewm[:])
            dix_sb = pp.tile([P, HROWS * 3 // 16], I16, tag="dix")
            nc.sync.dma_start(dix_sb[:], dix[:])
            dwm_sb = pp.tile([P, GBLK * 3], F32, tag="dwm")
            nc.sync.dma_start(dwm_sb[:], dwm[:])
            g1 = pp.tile([1, C], F32, tag="g1")
            nc.sync.dma_start(g1[:], gb2[0:1, :])
            b1 = pp.tile([1, C], F32, tag="b1")
            nc.sync.dma_start(b1[:], gb2[1:2, :])

            # ---- gamma/beta broadcast across partitions via K=1 matmul
            ones1 = pp.tile([1, P], F32, tag="ones1")
            nc.vector.memset(ones1[:], 1.0)
            gbc = pp.tile([P, C], F32, tag="gbc")
            bbc = pp.tile([P, C], F32, tag="bbc")
            for src, dst in ((g1, gbc), (b1, bbc)):
                psb = psp.tile([P, C], F32, tag="psb")
                nc.tensor.matmul(
                    psb[:], lhsT=ones1[:], rhs=src[:],
                    start=True, stop=True,
                )
                nc.scalar.activation(dst[:], psb[:], AF.Copy)

            piv_p = pp.tile([P, 16, C], F32, tag="pivp")

            # ---- phase E: gather node features (pairs), layernorm, combine
            with tc.tile_pool(name="pe", bufs=1) as pe:
                g2 = pe.tile([P, EIB, 2 * C], F32, tag="g2")
                ncalls = 8
                per = NEIX // ncalls            # 768 idx per call
                nf_pairs = nf[:].rearrange("(n two) c -> n (two c)", two=2)
                for ci in range(ncalls):
                    nc.gpsimd.dma_gather(
                        g2[:, ci * (EIB // ncalls) : (ci + 1) * (EIB // ncalls), :],
                        nf_pairs,
                        eix_sb[:, ci * (per // 16) : (ci + 1) * (per // 16)],
                        num_idxs=per,
                        num_idxs_reg=per,
                        elem_size=2 * C,
                        queue_num=ci % NQUEUES,
                    )
                # ge = (1-par)*g2_lo + par*g2_hi  (exact: par is 0.0/1.0)
                parc = pe.tile([P, EIB], F32, tag="parc")
                nc.vector.tensor_scalar(
                    parc[:], epar_sb[:], -1.0, 1.0, op0=ALU.mult, op1=ALU.add
                )
                ge = pe.tile([P, EIB, C], F32, tag="ge")
                tsel = pe.tile([P, EIB, C], F32, tag="tsel")
                nc.vector.tensor_tensor(
                    ge[:], g2[:, :, 0:C],
                    parc[:, :, None].to_broadcast([P, EIB, C]), op=ALU.mult,
                )
                nc.vector.tensor_tensor(
                    tsel[:], g2[:, :, C : 2 * C],
                    epar_sb[:, :, None].to_broadcast([P, EIB, C]), op=ALU.mult,
                )
                nc.vector.tensor_tensor(ge[:], ge[:], tsel[:], op=ALU.add)
                # layernorm stats per gathered row
                mu = pe.tile([P, EIB], F32, tag="mu")
                nc.vector.reduce_sum(mu[:], ge[:], axis=AX.X)
                nc.vector.tensor_scalar(mu[:], mu[:], 1.0 / C, None, op0=ALU.mult)
                nc.vector.tensor_tensor(
                    ge[:], ge[:], mu[:, :, None].to_broadcast([P, EIB, C]),
                    op=ALU.subtract,
                )
                sq = pe.tile([P, EIB, C], F32, tag="sq")
                nc.scalar.square(sq[:], ge[:])
                vs = pe.tile([P, EIB], F32, tag="vs")
                nc.vector.reduce_sum(vs[:], sq[:], axis=AX.X)
                nc.vector.tensor_scalar(
                    vs[:], vs[:], 1.0 / C, 1e-5, op0=ALU.mult, op1=ALU.add
                )
                nc.scalar.sqrt(vs[:], vs[:])
                nc.vector.reciprocal(vs[:], vs[:])
                # fold rstd * weight into one multiplier
                nc.vector.tensor_tensor(vs[:], vs[:], ewm_sb[:], op=ALU.mult)
                nc.vector.tensor_tensor(
                    ge[:], ge[:], vs[:, :, None].to_broadcast([P, EIB, C]),
                    op=ALU.mult,
                )
                ger = ge[:].rearrange("p (u k) c -> p u k c", k=3)
                nc.vector.tensor_tensor(
                    piv_p[:], ger[:, :, 0, :], ger[:, :, 1, :], op=ALU.add
                )
                nc.vector.tensor_tensor(
                    piv_p[:], piv_p[:], ger[:, :, 2, :], op=ALU.add
                )
                # gamma/beta (weights sum to 1, so affine folds to the end);
                # final add narrows to bf16 to halve the phase-D gather volume.
                nc.vector.tensor_tensor(
                    piv_p[:], piv_p[:],
                    gbc[:, None, :].to_broadcast([P, 16, C]), op=ALU.mult,
                )
                piv_bf = pe.tile([P, 16, C], BF16, tag="pivbf")
                nc.vector.tensor_tensor(
                    piv_bf[:], piv_p[:],
                    bbc[:, None, :].to_broadcast([P, 16, C]), op=ALU.add,
                )
                piv_d = dp.tile([NPP, C], BF16, tag="pivd")
                nc.sync.dma_start(
                    piv_d[:].rearrange("(p u) c -> p u c", p=P), piv_bf[:]
                )

            # ---- phase D: gather piv rows, combine with decode weights.
            # Superblocks alternate between DVE and Pool (~2:1 ratio) so the
            # multiply/add passes run on both engines in parallel.
            with tc.tile_pool(name="pd", bufs=3) as pd:
                for s in range(NSB):
                    eng = nc.gpsimd if s % 3 == 2 else nc.vector
                    g = pd.tile([P, 3 * SB, C], BF16, tag="g")
                    nidx = 3 * SB * P          # 3072
                    nc.gpsimd.dma_gather(
                        g[:],
                        piv_d[:],
                        dix_sb[:, s * (nidx // 16) : (s + 1) * (nidx // 16)],
                        num_idxs=nidx,
                        num_idxs_reg=nidx,
                        elem_size=C,
                        queue_num=s % NQUEUES,
                        single_packet=False,  # >1024 descriptors per call
                    )
                    t = pd.tile([P, 3 * SB, C], F32, tag="t")
                    eng.tensor_tensor(
                        t[:], g[:],
                        dwm_sb[:, s * 3 * SB : (s + 1) * 3 * SB][:, :, None]
                        .to_broadcast([P, 3 * SB, C]),
                        op=ALU.mult,
                    )
                    tr = t[:].rearrange("p (u k) c -> p u k c", k=3)
                    o = pd.tile([P, SB, C], F32, tag="o")
                    eng.tensor_tensor(
                        o[:], tr[:, :, 0, :], tr[:, :, 1, :], op=ALU.add
                    )
                    eng.tensor_tensor(o[:], o[:], tr[:, :, 2, :], op=ALU.add)
                    nc.sync.dma_start(
                        outd[:, s * SB * C : (s + 1) * SB * C], o[:]
                    )
    nc.finalize()
    return nc


# ---------------------------------------------------------------------------
# Host-side marshalling
# ---------------------------------------------------------------------------

def _pad_coords(pos, n):
    out = np.full((n, 3), PADC, np.float32)
    out[: pos.shape[0]] = pos
    return out


def _aug_cand(pos):
    # rows x0, x1, x2, |x|^2
    return np.concatenate(
        [pos.T, (pos * pos).sum(-1)[None, :]], axis=0
    ).astype(np.float32)


def _aug_query(pos):
    # rows -2*y0, -2*y1, -2*y2, 1
    return np.concatenate(
        [(-2.0 * pos).T, np.ones((1, pos.shape[0]), np.float32)], axis=0
    ).astype(np.float32)


def _wrap16(x_stream):
    """[N] stream (position j) -> [128, N//16] wrapped int16 (replicated x8)."""
    w16 = x_stream.reshape(-1, 16).T  # [16, N//16]
    return np.ascontiguousarray(np.tile(w16, (8, 1)).astype(np.int16))


def _slot_layout(idx_qk):
    """[Q, 3] per-query data -> [P, 3*Q/P] slot layout: out[p, 3g+k] = in[g*128+p, k]."""
    q = idx_qk.shape[0]
    g = q // P
    return np.ascontiguousarray(
        idx_qk.reshape(g, P, 3).transpose(1, 0, 2).reshape(P, 3 * g)
    )


def launch1_inputs(pm, pv):
    """Per-core input dicts for the knn launch."""
    cm = _pad_coords(pm, NMP)
    cp = _pad_coords(pv, NPP)
    ce4 = _aug_cand(cm)
    cd4 = _aug_cand(cp)
    qe4g = _aug_query(cp)
    qeng = (-(cp * cp).sum(-1)).astype(np.float32)
    qd4g = _aug_query(cm)
    qdng = (-(cm * cm).sum(-1)).astype(np.float32)

    in1 = []
    for c in range(NCORES):
        in1.append(
            dict(
                ce4=ce4,
                cd4=cd4,
                qe4=np.ascontiguousarray(qe4g[:, c * QE : (c + 1) * QE]),
                qen=np.ascontiguousarray(
                    qeng[c * QE : (c + 1) * QE].reshape(EB, P).T
                ),
                qd4=np.ascontiguousarray(qd4g[:, c * QD : (c + 1) * QD]),
                qdn=np.ascontiguousarray(
                    qdng[c * QD : (c + 1) * QD].reshape(DB, P).T
                ),
            )
        )
    return in1


def launch1_post(r1):
    """Assemble per-core knn outputs into global idx/weight tables."""
    enc_idx = np.zeros((NPP, 3), np.int64)
    enc_w = np.zeros((NPP, 3), np.float32)
    dec_idx = np.zeros((NMP, 3), np.int64)
    dec_w = np.zeros((NMP, 3), np.float32)
    for c in range(NCORES):
        ei = np.asarray(r1[c]["ei"])  # [P, EB, 3] f32
        ew = np.asarray(r1[c]["ew"])
        di = np.asarray(r1[c]["di"])  # [P, DB, 3] u16
        dw = np.asarray(r1[c]["dw"])
        enc_idx[c * QE : (c + 1) * QE] = np.rint(
            ei.transpose(1, 0, 2).reshape(QE, 3)
        ).astype(np.int64)
        enc_w[c * QE : (c + 1) * QE] = ew.transpose(1, 0, 2).reshape(QE, 3)
        dec_idx[c * QD : (c + 1) * QD] = (
            di.transpose(1, 0, 2).reshape(QD, 3).astype(np.int64)
        )
        dec_w[c * QD : (c + 1) * QD] = dw.transpose(1, 0, 2).reshape(QD, 3)
    return enc_idx, enc_w, dec_idx, dec_w


def launch2_inputs(nf, gam, bet, enc_idx, enc_w, dec_idx, dec_w):
    """Per-core input dicts for the interp launch."""
    # Padded pivots may select padded mesh candidates (idx >= NM); their piv
    # rows are only ever consumed by padded decode queries whose outputs are
    # discarded, so point them at row 0.
    enc_idx = np.where(enc_idx >= NM, 0, enc_idx)
    # Encode gather marshalling (identical on the two cores of a batch).
    # Slot (p, i=3u+k) <- pivot 16p+u, neighbor k.
    piv_ids = (16 * np.arange(P)[:, None] + np.arange(16)[None, :]).reshape(-1)
    e_idx_slot = enc_idx[piv_ids].reshape(P, EIB)          # [P, 48]
    e_w_slot = enc_w[piv_ids].reshape(P, EIB).astype(np.float32)
    e_pair_slot = (e_idx_slot >> 1).astype(np.int16)
    e_par_slot = (e_idx_slot & 1).astype(np.float32)
    # stream j = i*128 + p -> value slot[p, i]
    eix_h = _wrap16(e_pair_slot.T.reshape(-1))
    gb2_h = np.ascontiguousarray(np.stack([gam, bet]).astype(np.float32))

    in2 = []
    for c in range(NCORES):
        b, h = divmod(c, 2)
        rows = slice(h * HROWS, (h + 1) * HROWS)
        d_idx_slot = _slot_layout(dec_idx[rows]).astype(np.int16)  # [P, 480]
        d_w_slot = _slot_layout(dec_w[rows]).astype(np.float32)
        dix_h = _wrap16(d_idx_slot.T.reshape(-1))
        in2.append(
            dict(
                nf=np.ascontiguousarray(nf[b * NM : (b + 1) * NM]),
                gb2=gb2_h,
                eix=eix_h,
                epar=np.ascontiguousarray(e_par_slot),
                ewm=np.ascontiguousarray(e_w_slot),
                dix=dix_h,
                dwm=d_w_slot,
            )
        )
    return in2


def launch2_post(r2):
    out = np.empty((B * NM, C), np.float32)
    for c in range(NCORES):
        b, h = divmod(c, 2)
        od = np.asarray(r2[c]["outd"]).reshape(P, GBLK, C)
        block = od.transpose(1, 0, 2).reshape(HROWS, C)
        lo = h * HROWS
        n_real = min(NM - lo, HROWS)
        out[b * NM + lo : b * NM + lo + n_real] = block[:n_real]
    return out


def kernel(node_features, gamma, beta, position_mesh, position_pivotal, batch_size):
    nf = np.ascontiguousarray(np.asarray(node_features, dtype=np.float32))
    gam = np.asarray(gamma, dtype=np.float32).reshape(C)
    bet = np.asarray(beta, dtype=np.float32).reshape(C)
    pm = np.asarray(position_mesh, dtype=np.float32)
    pv = np.asarray(position_pivotal, dtype=np.float32)
    bsz = int(batch_size)
    assert bsz == B and pm.shape[0] == NM and pv.shape[0] == NP

    in1 = launch1_inputs(pm, pv)
    rr1 = run_bass_kernel_spmd(_knn_program(), in1, list(range(NCORES)))
    enc_idx, enc_w, dec_idx, dec_w = launch1_post(rr1.results)

    in2 = launch2_inputs(nf, gam, bet, enc_idx, enc_w, dec_idx, dec_w)
    rr2 = run_bass_kernel_spmd(_interp_program(), in2, list(range(NCORES)))
    LAST_EXEC_NS.clear()
    LAST_EXEC_NS.extend([rr1.exec_time_ns, rr2.exec_time_ns])
    return launch2_post(rr2.results)
